# revision 41
# baseline (speedup 1.0000x reference)
"""Multi-head causal attention (B=4, T=2048, d_model=1024, 16 heads) on 8 trn2 cores.

Sharding: core c = (batch b = c//2, head-group g = c%2 of 8 heads) — data
parallel on B, tensor parallel on heads, per the problem's sharding hint.
Per core: QKV projection for its batch/head-group (Q,K produced in [d, t]
layout, V in [t, d] with a ones-column per head); causal attention with
S^T-orientation matmuls (2 heads row-packed in the 128-row PE array since
d_head=64), exp on ScalarE (the 1/sqrt(d) scale folded into the ACT affine),
M=65 AV matmuls whose 65th output row IS the softmax denominator,
normalization via K=1 broadcast matmuls + reciprocal_approx_fast; partial
output projection (contract over this group's 512 y-dims) + bias/2; then an
on-device ReduceScatter(add) over core pairs (split into 4 token slices to
overlap with the projection) so each core emits disjoint 256-token slices of
the final output. Host only shards inputs / concatenates outputs.

All matmuls bf16 with f32 PSUM accumulation; softmax in f32 (no max
subtraction needed: scores ~ N(0,1), |s| < ~7, exp is safe in f32).
Measured end-to-end absmax-relative error vs the f32 reference: ~6e-3.
HW exec time: ~416 us (profiled via neuron-profile NTFF, slowest core).
"""

import sys
import types

import numpy as np
import ml_dtypes

import concourse.bass as bass
import concourse.bacc as bacc
import concourse.mybir as mybir
import concourse.tile as tile
from concourse.bass_utils import run_bass_kernel_spmd


def _install_ntff_hook():
    """Register the axon NTFF profile hook if the image's antenv lacks it.

    trn_boot degrades silently when `antenv.axon_hooks` is missing, which
    makes any run_bass_kernel_spmd(trace=True) (e.g. BASS_TRACE=1) crash
    with ModuleNotFoundError instead of profiling. Supply the module and
    wire the ctypes hook so tracing works.
    """
    if "antenv.axon_hooks" in sys.modules:
        return
    try:
        m = types.ModuleType("antenv.axon_hooks")
        m._hook = None
        m.set_axon_ntff_profile_hook = lambda h: setattr(m, "_hook", h)
        m.get_axon_ntff_profile_hook = lambda: m._hook
        import antenv
        from trn_agent_boot.trn_boot import _ntff_profile_via_ctypes
        m._hook = _ntff_profile_via_ctypes("/opt/axon/libaxon_pjrt.so")
        sys.modules["antenv.axon_hooks"] = m
        antenv.axon_hooks = m
    except Exception:
        pass


_install_ntff_hook()

dt = mybir.dt

N_CORES = 8
B, T, C = 4, 2048, 1024
H, DH = 16, 64
HPC = 8            # heads per core (head-group)
GDIM = HPC * DH    # 512 = y-dims owned by one core
NPACK = 4          # head pairs per core
NCHUNK = 4         # q chunks of 512
QC = 512           # q chunk width
KT = 128           # k tile width
SCALE = DH ** -0.5


def build_nc():
    nc = bacc.Bacc("TRN2", target_bir_lowering=False, debug=False,
                   num_devices=N_CORES)

    xT = nc.dram_tensor("xT", [C, T], dt.bfloat16, kind="ExternalInput")
    wT = nc.dram_tensor("wT", [C, 3 * GDIM], dt.bfloat16, kind="ExternalInput")
    wpT = nc.dram_tensor("wpT", [GDIM, C], dt.bfloat16, kind="ExternalInput")
    biasb = nc.dram_tensor("biasb", [128, C], dt.float32, kind="ExternalInput")
    masks = nc.dram_tensor("masks", [128, 4 * 1024], dt.bfloat16, kind="ExternalInput")
    ones = nc.dram_tensor("ones", [128, 64], dt.bfloat16, kind="ExternalInput")
    out_ext = nc.dram_tensor("out_ext", [T // 2, C], dt.bfloat16, kind="ExternalOutput")

    with tile.TileContext(nc) as tc:
        with (
            tc.tile_pool(name="persist", bufs=1) as pp,
            tc.tile_pool(name="work", bufs=4) as wp,
            tc.tile_pool(name="outp", bufs=3) as op,
            tc.tile_pool(name="psum", bufs=2, space="PSUM") as pps,
            tc.tile_pool(name="dram", bufs=1, space="DRAM") as dp,
        ):
            # ---- load inputs (spread across DGE queues: 4 engines) ----
            qs = [nc.sync]
            xT_sb, wT_sb, wpT_sb = [], [], []
            for i in range(8):
                t = pp.tile([128, T], dt.bfloat16, tag=f"xT{i}", name=f"xT{i}")
                qs[0].dma_start(t[:], xT[128 * i:128 * (i + 1), :])
                xT_sb.append(t)
            for i in range(8):
                t = pp.tile([128, 3 * GDIM], dt.bfloat16, tag=f"wT{i}", name=f"wT{i}")
                qs[0].dma_start(t[:], wT[128 * i:128 * (i + 1), :])
                wT_sb.append(t)
            for i in range(4):
                t = pp.tile([128, C], dt.bfloat16, tag=f"wpT{i}", name=f"wpT{i}")
                qs[0].dma_start(t[:], wpT[128 * i:128 * (i + 1), :])
                wpT_sb.append(t)
            bias_sb = pp.tile([128, C], dt.float32, tag="bias")
            nc.sync.dma_start(bias_sb[:], biasb[:])
            mask_sb = pp.tile([128, 4 * 1024], dt.bfloat16, tag="masks")
            nc.sync.dma_start(mask_sb[:], masks[:])
            ones_sb = pp.tile([128, 64], dt.bfloat16, tag="ones")
            nc.sync.dma_start(ones_sb[:], ones[:])

            # ---- PE warmup: ~10us of junk matmuls during the input DMA so
            #      the HAM clock-gate is at 8/8 before real work starts ----
            junk = pp.tile([128, 640], dt.bfloat16, tag="junk")
            nc.vector.memset(junk[:], 1.0)
            jps = pps.tile([128, 1024], dt.float32, tag="big", bufs=3)
            for r in range(24):
                nc.tensor.matmul(
                    jps[:, 0:512], lhsT=junk[:, 0:128], rhs=junk[:, 128:640],
                    start=(r == 0), stop=(r == 23))

            # ---- V = x @ Wv  ([t, d] layout), 16 token tiles ----
            # per head h: cols [65h:65h+64] = V data, col 65h+64 = 1.0 so the
            # M=65 AV matmul emits the softmax denominator as its 65th row
            v_sb = []
            for tt in range(16):
                v = pp.tile([128, 8 * 65], dt.bfloat16, tag=f"v{tt}",
                            name=f"v{tt}")
                ones_cols = v.rearrange("p (h e) -> p h e", e=65)[:, :, 64:65]
                nc.gpsimd.memset(ones_cols, 1.0)
                v_sb.append(v)

            def emit_v_quarter(vq):
                for half in range(2 * vq, 2 * vq + 2):
                    ps = pps.tile([128, 1024], dt.float32, tag="big", bufs=3)
                    for s in range(2):
                        tt = 2 * half + s
                        for ck in range(8):
                            nc.tensor.matmul(
                                ps[:, 512 * s:512 * (s + 1)],
                                lhsT=xT_sb[ck][:, 128 * tt:128 * (tt + 1)],
                                rhs=wT_sb[ck][:, 2 * GDIM:3 * GDIM],
                                start=(ck == 0), stop=(ck == 7),
                            )
                    for s in range(2):
                        tt = 2 * half + s
                        dst = v_sb[tt].rearrange("p (h e) -> p h e", e=65)[:, :, 0:64]
                        src = ps[:, 512 * s:512 * (s + 1)].rearrange(
                            "p (h d) -> p h d", d=64)
                        nc.vector.tensor_copy(dst, src)

            # ---- Q^T / K^T projections (emitted per pack, interleaved with
            #      the previous pack's attention for PE density) ----
            def alloc_qk(p):
                return (pp.tile([128, T], dt.bfloat16, tag=f"qT{p}", name=f"qT{p}"),
                        pp.tile([128, T], dt.bfloat16, tag=f"kT{p}", name=f"kT{p}"))

            def emit_qk_quarter(p, dsts, quarter):
                # quarter 0,1 -> Q halves; 2,3 -> K halves
                kind = quarter // 2
                halft = quarter % 2
                fofs = 128 * p + GDIM * kind
                dst = dsts[kind]
                ps = pps.tile([128, 1024], dt.float32, tag="big", bufs=3)
                for s in range(2):
                    for ck in range(8):
                        nc.tensor.matmul(
                            ps[:, 512 * s:512 * (s + 1)],
                            lhsT=wT_sb[ck][:, fofs:fofs + 128],
                            rhs=xT_sb[ck][:, 1024 * halft + 512 * s:
                                           1024 * halft + 512 * (s + 1)],
                            start=(ck == 0), stop=(ck == 7),
                        )
                nc.vector.tensor_copy(
                    dst[:, 1024 * halft:1024 * (halft + 1)], ps[:])

            y_sb = {}

            def emit_attention_chunk(p, qT, kT, c):
                    jmax = 4 * c + 3
                    # one 2-bank tile: bank0 = y pair, bank1 = l rows then bcast
                    ypl = pps.tile([128, 1024], dt.float32, tag="ypl", bufs=1)
                    for j in range(jmax + 1):
                        ps = pps.tile([128, 1024], dt.float32, tag="big", bufs=3)
                        nc.tensor.matmul(
                            ps[:, 0:QC],
                            lhsT=kT[0:64, KT * j:KT * (j + 1)],
                            rhs=qT[0:64, QC * c:QC * (c + 1)],
                            start=True, stop=True,
                        )
                        nc.tensor.matmul(
                            ps[:, QC:2 * QC],
                            lhsT=kT[64:128, KT * j:KT * (j + 1)],
                            rhs=qT[64:128, QC * c:QC * (c + 1)],
                            start=True, stop=True,
                        )
                        pt = wp.tile([128, 1024], dt.bfloat16, tag="pt", bufs=12)
                        nc.scalar.activation(
                            pt[:], ps[:], mybir.ActivationFunctionType.Exp,
                            scale=SCALE)
                        if j >= 4 * c:  # diagonal: zero the upper triangle
                            r = j - 4 * c
                            nc.vector.tensor_mul(
                                pt[:], pt[:], mask_sb[:, 1024 * r:1024 * (r + 1)])
                        first, last = (j == 0), (j == jmax)
                        # M=65 AV per head: rows 0:64 = y^T, row 64 = l
                        # (bank0 = head 2p, bank1 = head 2p+1)
                        for h in range(2):
                            hh = 2 * p + h
                            nc.tensor.matmul(
                                ypl[0:65, QC * h:QC * (h + 1)],
                                lhsT=v_sb[j][:, 65 * hh:65 * hh + 65],
                                rhs=pt[:, QC * h:QC * (h + 1)],
                                start=first, stop=last,
                            )
                    # evacuate PSUM; the h2 y block must end up on partitions
                    # 64:128 which only a DMA can do (cross-partition move)
                    lb = wp.tile([128, 2 * QC], dt.bfloat16, tag="lb", bufs=3)
                    ycp = wp.tile([128, QC], dt.float32, tag="ycp", bufs=3)
                    st2 = wp.tile([128, QC], dt.float32, tag="st2", bufs=3)
                    with tc.high_priority():  # these gate the ypl slot release
                        nc.vector.tensor_copy(lb[64:65, :], ypl[64:65, :])
                        nc.vector.tensor_copy(ycp[0:64, :], ypl[0:64, 0:QC])
                        nc.vector.tensor_copy(st2[0:64, :], ypl[0:64, QC:2 * QC])
                        nc.sync.dma_start(ycp[64:128, :], st2[0:64, :])
                    # broadcast l across the d rows with K=1 matmuls
                    bb = pps.tile([128, 1024], dt.float32, tag="big", bufs=3)
                    nc.tensor.matmul(
                        bb[0:64, 0:QC], lhsT=ones_sb[64:65, :],
                        rhs=lb[64:65, 0:QC],
                        start=True, stop=True, tile_position=(64, 0))
                    nc.tensor.matmul(
                        bb[64:128, 0:QC], lhsT=ones_sb[64:65, :],
                        rhs=lb[64:65, QC:2 * QC],
                        start=True, stop=True, tile_position=(64, 64))
                    rb = wp.tile([128, QC], dt.float32, tag="rb", bufs=3)
                    nc.vector.reciprocal_approx_fast(rb[:], bb[:, 0:QC])
                    yt = pp.tile([128, QC], dt.bfloat16, tag=f"y{p}_{c}",
                                 name=f"y{p}_{c}")
                    nc.vector.tensor_mul(yt[:], ycp[:], rb[:])
                    y_sb[(p, c)] = yt

            # ---- partial projection + bias/2, ReduceScatter per 512-slice ----
            # bf16 partials/outputs: halves collective bytes; host casts back
            def emit_proj_slice(sl):
                part = dp.tile([QC, C], dt.bfloat16, name=f"part{sl}")
                for tt in range(4 * sl, 4 * sl + 4):
                    c = tt // 4
                    ps = pps.tile([128, 1024], dt.float32, tag="big", bufs=3)
                    for oc in range(2):
                        for p in range(NPACK):
                            nc.tensor.matmul(
                                ps[:, 512 * oc:512 * (oc + 1)],
                                lhsT=y_sb[(p, c)][:, 128 * (tt % 4):
                                                  128 * (tt % 4 + 1)],
                                rhs=wpT_sb[p][:, 512 * oc:512 * (oc + 1)],
                                start=(p == 0), stop=(p == 3),
                            )
                    os_ = op.tile([128, C], dt.bfloat16, tag="osb")
                    with nc.allow_low_precision("bf16 partial + pairwise reduce"):
                        nc.vector.tensor_add(os_[:], ps[:], bias_sb[:])
                    nc.sync.dma_start(
                        part[128 * (tt % 4):128 * (tt % 4 + 1), :], os_[:])
                rs_out = dp.tile([QC // 2, C], dt.bfloat16, name=f"rs{sl}")
                nc.gpsimd.collective_compute(
                    "ReduceScatter",
                    mybir.AluOpType.add,
                    replica_groups=[[0, 1], [2, 3], [4, 5], [6, 7]],
                    ins=[part[:]],
                    outs=[rs_out[:]],
                )
                nc.sync.dma_start(
                    out_ext[256 * sl:256 * (sl + 1), :], rs_out[:])

            # ---- emission schedule: QK0, V, then attention chunks with the
            #      next pack's QK quarters (or proj slices) interleaved ----
            # gap-filler blocks (V/QK/proj) are demoted in scheduler
            # priority so the attention S-matmul -> exp chain never starves;
            # emission ORDER still defines the data dependencies
            LOW = -1_000_000
            qk = {0: alloc_qk(0)}
            for q in (0, 2):        # Q half0, K half0: all chunks 0-1 need
                emit_qk_quarter(0, qk[0], q)
            # pack 0 (V interleaved), pack 1, then packs 2+3 chunk-interleaved
            # so proj slices (gated on pack 3's chunks) spread over the tail
            for c in range(NCHUNK):
                if c == 2:  # halves 1 needed from chunk 2 on
                    for q in (1, 3):
                        emit_qk_quarter(0, qk[0], q)
                with tc.high_priority(offset=LOW):
                    emit_v_quarter(c)
                emit_attention_chunk(0, *qk[0], c)
            qk[1] = alloc_qk(1)
            with tc.high_priority(offset=LOW):
                for q in (0, 2, 1, 3):
                    emit_qk_quarter(1, qk[1], q)
            for c in range(NCHUNK):
                emit_attention_chunk(1, *qk[1], c)
            for p in (2, 3):
                qk[p] = alloc_qk(p)
                with tc.high_priority(offset=LOW):
                    for q in (0, 2, 1, 3):
                        emit_qk_quarter(p, qk[p], q)
            # reversed chunk order: the 16-k-tile chunk 3 runs first and the
            # 4-k-tile chunk 0 last, so the final attention->proj->RS serial
            # tail is as short as possible
            for c in (3, 2, 1, 0):
                emit_attention_chunk(2, *qk[2], c)
                emit_attention_chunk(3, *qk[3], c)
                with tc.high_priority(offset=LOW):
                    emit_proj_slice(c)

    nc.compile()
    return nc


_NC = None


def _get_nc():
    global _NC
    if _NC is None:
        _NC = build_nc()
    return _NC


def _make_in_maps(x, w_qkv, w_proj, b_proj):
    bf16 = ml_dtypes.bfloat16
    # causal masks for the 4 diagonal k-tiles of a 512-q chunk, both head
    # halves identical: mask_r[ki, qi] = qi >= 128*r + ki
    qi = np.arange(QC)[None, :]
    ki = np.arange(128)[:, None]
    mk = np.concatenate(
        [np.tile((qi >= 128 * r + ki), (1, 2)) for r in range(4)], axis=1)
    masks = mk.astype(bf16)
    ones = np.ones((128, 64), dtype=bf16)
    biasb = np.tile(b_proj.astype(np.float32)[None, :] * 0.5, (128, 1))

    in_maps = []
    for c in range(N_CORES):
        b, g = c // 2, c % 2
        xTc = np.ascontiguousarray(x[b].T).astype(bf16)
        rows = []
        for blk in range(3):  # q, k, v rows of w_qkv for this head group
            base = blk * C + g * GDIM
            rows.append(w_qkv[base:base + GDIM, :])
        wTc = np.ascontiguousarray(np.concatenate(rows, axis=0).T).astype(bf16)
        wpTc = np.ascontiguousarray(
            w_proj[:, g * GDIM:(g + 1) * GDIM].T).astype(bf16)
        in_maps.append({
            "xT": xTc, "wT": wTc, "wpT": wpTc,
            "biasb": biasb, "masks": masks, "ones": ones,
        })
    return in_maps


def kernel(x, w_qkv, w_proj, b_proj):
    x = np.asarray(x, dtype=np.float32)
    w_qkv = np.asarray(w_qkv, dtype=np.float32)
    w_proj = np.asarray(w_proj, dtype=np.float32)
    b_proj = np.asarray(b_proj, dtype=np.float32)

    nc = _get_nc()
    in_maps = _make_in_maps(x, w_qkv, w_proj, b_proj)
    res = run_bass_kernel_spmd(nc, in_maps, list(range(N_CORES)))

    # each 512-token-slice ReduceScatter scatters over the pair in rank
    # order (256 tokens each); host casts bf16 results back to f32
    out = np.empty((B, T, C), dtype=np.float32)
    for cc in range(N_CORES):
        b, g = cc // 2, cc % 2
        r = res.results[cc]["out_ext"].astype(np.float32)
        for sl in range(4):
            t0 = 512 * sl + 256 * g
            out[b, t0:t0 + 256, :] = r[256 * sl:256 * (sl + 1), :]
    return out


# revision 42
# speedup vs baseline: 1.0021x; 1.0021x over previous
"""Multi-head causal attention (B=4, T=2048, d_model=1024, 16 heads) on 8 trn2 cores.

Sharding: core c = (batch b = c//2, head-group g = c%2 of 8 heads) — data
parallel on B, tensor parallel on heads, per the problem's sharding hint.
Per core: QKV projection for its batch/head-group (Q,K produced in [d, t]
layout, V in [t, d] with a ones-column per head); causal attention with
S^T-orientation matmuls (2 heads row-packed in the 128-row PE array since
d_head=64), exp on ScalarE (the 1/sqrt(d) scale folded into the ACT affine),
M=65 AV matmuls whose 65th output row IS the softmax denominator,
normalization via K=1 broadcast matmuls + reciprocal_approx_fast; partial
output projection (contract over this group's 512 y-dims) + bias/2; then an
on-device ReduceScatter(add) over core pairs (split into 4 token slices to
overlap with the projection) so each core emits disjoint 256-token slices of
the final output. Host only shards inputs / concatenates outputs.

All matmuls bf16 with f32 PSUM accumulation; softmax in f32 (no max
subtraction needed: scores ~ N(0,1), |s| < ~7, exp is safe in f32).
Measured end-to-end absmax-relative error vs the f32 reference: ~6e-3.
HW exec time: ~416 us (profiled via neuron-profile NTFF, slowest core).
"""

import sys
import types

import numpy as np
import ml_dtypes

import concourse.bass as bass
import concourse.bacc as bacc
import concourse.mybir as mybir
import concourse.tile as tile
from concourse.bass_utils import run_bass_kernel_spmd


def _install_ntff_hook():
    """Register the axon NTFF profile hook if the image's antenv lacks it.

    trn_boot degrades silently when `antenv.axon_hooks` is missing, which
    makes any run_bass_kernel_spmd(trace=True) (e.g. BASS_TRACE=1) crash
    with ModuleNotFoundError instead of profiling. Supply the module and
    wire the ctypes hook so tracing works.
    """
    if "antenv.axon_hooks" in sys.modules:
        return
    try:
        m = types.ModuleType("antenv.axon_hooks")
        m._hook = None
        m.set_axon_ntff_profile_hook = lambda h: setattr(m, "_hook", h)
        m.get_axon_ntff_profile_hook = lambda: m._hook
        import antenv
        from trn_agent_boot.trn_boot import _ntff_profile_via_ctypes
        m._hook = _ntff_profile_via_ctypes("/opt/axon/libaxon_pjrt.so")
        sys.modules["antenv.axon_hooks"] = m
        antenv.axon_hooks = m
    except Exception:
        pass


_install_ntff_hook()

dt = mybir.dt

N_CORES = 8
B, T, C = 4, 2048, 1024
H, DH = 16, 64
HPC = 8            # heads per core (head-group)
GDIM = HPC * DH    # 512 = y-dims owned by one core
NPACK = 4          # head pairs per core
NCHUNK = 4         # q chunks of 512
QC = 512           # q chunk width
KT = 128           # k tile width
SCALE = DH ** -0.5


def build_nc():
    nc = bacc.Bacc("TRN2", target_bir_lowering=False, debug=False,
                   num_devices=N_CORES)

    xT = nc.dram_tensor("xT", [C, T], dt.bfloat16, kind="ExternalInput")
    wT = nc.dram_tensor("wT", [C, 3 * GDIM], dt.bfloat16, kind="ExternalInput")
    wpT = nc.dram_tensor("wpT", [GDIM, C], dt.bfloat16, kind="ExternalInput")
    biasb = nc.dram_tensor("biasb", [128, C], dt.float32, kind="ExternalInput")
    masks = nc.dram_tensor("masks", [128, 4 * 1024], dt.bfloat16, kind="ExternalInput")
    ones = nc.dram_tensor("ones", [128, 64], dt.bfloat16, kind="ExternalInput")
    out_ext = nc.dram_tensor("out_ext", [T // 2, C], dt.bfloat16, kind="ExternalOutput")

    with tile.TileContext(nc) as tc:
        with (
            tc.tile_pool(name="persist", bufs=1) as pp,
            tc.tile_pool(name="work", bufs=4) as wp,
            tc.tile_pool(name="outp", bufs=3) as op,
            tc.tile_pool(name="psum", bufs=2, space="PSUM") as pps,
            tc.tile_pool(name="dram", bufs=1, space="DRAM") as dp,
        ):
            # ---- load inputs (spread across DGE queues: 4 engines) ----
            qs = [nc.sync]
            xT_sb, wT_sb, wpT_sb = [], [], []
            for i in range(8):
                t = pp.tile([128, T], dt.bfloat16, tag=f"xT{i}", name=f"xT{i}")
                qs[0].dma_start(t[:], xT[128 * i:128 * (i + 1), :])
                xT_sb.append(t)
            for i in range(8):
                t = pp.tile([128, 3 * GDIM], dt.bfloat16, tag=f"wT{i}", name=f"wT{i}")
                qs[0].dma_start(t[:], wT[128 * i:128 * (i + 1), :])
                wT_sb.append(t)
            for i in range(4):
                t = pp.tile([128, C], dt.bfloat16, tag=f"wpT{i}", name=f"wpT{i}")
                qs[0].dma_start(t[:], wpT[128 * i:128 * (i + 1), :])
                wpT_sb.append(t)
            bias_sb = pp.tile([128, C], dt.float32, tag="bias")
            nc.sync.dma_start(bias_sb[:], biasb[:])
            mask_sb = pp.tile([128, 4 * 1024], dt.bfloat16, tag="masks")
            nc.sync.dma_start(mask_sb[:], masks[:])
            ones_sb = pp.tile([128, 64], dt.bfloat16, tag="ones")
            nc.sync.dma_start(ones_sb[:], ones[:])

            # ---- PE warmup: ~10us of junk matmuls during the input DMA so
            #      the HAM clock-gate is at 8/8 before real work starts ----
            junk = pp.tile([128, 640], dt.bfloat16, tag="junk")
            nc.vector.memset(junk[:], 1.0)
            jps = pps.tile([128, 1024], dt.float32, tag="big", bufs=3)
            for r in range(24):
                nc.tensor.matmul(
                    jps[:, 0:512], lhsT=junk[:, 0:128], rhs=junk[:, 128:640],
                    start=(r == 0), stop=(r == 23))

            # ---- V = x @ Wv  ([t, d] layout), 16 token tiles ----
            # per head h: cols [65h:65h+64] = V data, col 65h+64 = 1.0 so the
            # M=65 AV matmul emits the softmax denominator as its 65th row
            v_sb = []
            for tt in range(16):
                v = pp.tile([128, 8 * 65], dt.bfloat16, tag=f"v{tt}",
                            name=f"v{tt}")
                ones_cols = v.rearrange("p (h e) -> p h e", e=65)[:, :, 64:65]
                nc.gpsimd.memset(ones_cols, 1.0)
                v_sb.append(v)

            def emit_v_quarter(vq):
                for half in range(2 * vq, 2 * vq + 2):
                    ps = pps.tile([128, 1024], dt.float32, tag="big", bufs=3)
                    for s in range(2):
                        tt = 2 * half + s
                        for ck in range(8):
                            nc.tensor.matmul(
                                ps[:, 512 * s:512 * (s + 1)],
                                lhsT=xT_sb[ck][:, 128 * tt:128 * (tt + 1)],
                                rhs=wT_sb[ck][:, 2 * GDIM:3 * GDIM],
                                start=(ck == 0), stop=(ck == 7),
                            )
                    for s in range(2):
                        tt = 2 * half + s
                        dst = v_sb[tt].rearrange("p (h e) -> p h e", e=65)[:, :, 0:64]
                        src = ps[:, 512 * s:512 * (s + 1)].rearrange(
                            "p (h d) -> p h d", d=64)
                        nc.vector.tensor_copy(dst, src)

            # ---- Q^T / K^T projections (emitted per pack, interleaved with
            #      the previous pack's attention for PE density) ----
            def alloc_qk(p):
                return (pp.tile([128, T], dt.bfloat16, tag=f"qT{p}", name=f"qT{p}"),
                        pp.tile([128, T], dt.bfloat16, tag=f"kT{p}", name=f"kT{p}"))

            def emit_qk_quarter(p, dsts, quarter):
                # quarter 0,1 -> Q halves; 2,3 -> K halves
                kind = quarter // 2
                halft = quarter % 2
                fofs = 128 * p + GDIM * kind
                dst = dsts[kind]
                ps = pps.tile([128, 1024], dt.float32, tag="big", bufs=3)
                for s in range(2):
                    for ck in range(8):
                        nc.tensor.matmul(
                            ps[:, 512 * s:512 * (s + 1)],
                            lhsT=wT_sb[ck][:, fofs:fofs + 128],
                            rhs=xT_sb[ck][:, 1024 * halft + 512 * s:
                                           1024 * halft + 512 * (s + 1)],
                            start=(ck == 0), stop=(ck == 7),
                        )
                nc.vector.tensor_copy(
                    dst[:, 1024 * halft:1024 * (halft + 1)], ps[:])

            y_sb = {}

            def emit_attention_chunk(p, qT, kT, c):
                    jmax = 4 * c + 3
                    # one 2-bank tile: bank0 = y pair, bank1 = l rows then bcast
                    ypl = pps.tile([128, 1024], dt.float32, tag="ypl", bufs=1)
                    for j in range(jmax + 1):
                        ps = pps.tile([128, 1024], dt.float32, tag="big", bufs=3)
                        nc.tensor.matmul(
                            ps[:, 0:QC],
                            lhsT=kT[0:64, KT * j:KT * (j + 1)],
                            rhs=qT[0:64, QC * c:QC * (c + 1)],
                            start=True, stop=True,
                        )
                        nc.tensor.matmul(
                            ps[:, QC:2 * QC],
                            lhsT=kT[64:128, KT * j:KT * (j + 1)],
                            rhs=qT[64:128, QC * c:QC * (c + 1)],
                            start=True, stop=True,
                        )
                        pt = wp.tile([128, 1024], dt.bfloat16, tag="pt", bufs=12)
                        nc.scalar.activation(
                            pt[:], ps[:], mybir.ActivationFunctionType.Exp,
                            scale=SCALE)
                        if j >= 4 * c:  # diagonal: zero the upper triangle
                            r = j - 4 * c
                            nc.vector.tensor_mul(
                                pt[:], pt[:], mask_sb[:, 1024 * r:1024 * (r + 1)])
                        first, last = (j == 0), (j == jmax)
                        # M=65 AV per head: rows 0:64 = y^T, row 64 = l
                        # (bank0 = head 2p, bank1 = head 2p+1)
                        for h in range(2):
                            hh = 2 * p + h
                            nc.tensor.matmul(
                                ypl[0:65, QC * h:QC * (h + 1)],
                                lhsT=v_sb[j][:, 65 * hh:65 * hh + 65],
                                rhs=pt[:, QC * h:QC * (h + 1)],
                                start=first, stop=last,
                            )
                    # evacuate PSUM; the h2 y block must end up on partitions
                    # 64:128 which only a DMA can do (cross-partition move)
                    lb = wp.tile([128, 2 * QC], dt.bfloat16, tag="lb", bufs=3)
                    ycp = wp.tile([128, QC], dt.float32, tag="ycp", bufs=3)
                    st2 = wp.tile([128, QC], dt.float32, tag="st2", bufs=3)
                    with tc.high_priority():  # these gate the ypl slot release
                        nc.vector.tensor_copy(lb[64:65, :], ypl[64:65, :])
                        nc.vector.tensor_copy(ycp[0:64, :], ypl[0:64, 0:QC])
                        nc.vector.tensor_copy(st2[0:64, :], ypl[0:64, QC:2 * QC])
                        nc.sync.dma_start(ycp[64:128, :], st2[0:64, :])
                    # broadcast l across the d rows with K=1 matmuls
                    bb = pps.tile([128, 1024], dt.float32, tag="big", bufs=3)
                    nc.tensor.matmul(
                        bb[0:64, 0:QC], lhsT=ones_sb[64:65, :],
                        rhs=lb[64:65, 0:QC],
                        start=True, stop=True, tile_position=(64, 0))
                    nc.tensor.matmul(
                        bb[64:128, 0:QC], lhsT=ones_sb[64:65, :],
                        rhs=lb[64:65, QC:2 * QC],
                        start=True, stop=True, tile_position=(64, 64))
                    rb = wp.tile([128, QC], dt.float32, tag="rb", bufs=3)
                    nc.vector.reciprocal_approx_fast(rb[:], bb[:, 0:QC])
                    yt = pp.tile([128, QC], dt.bfloat16, tag=f"y{p}_{c}",
                                 name=f"y{p}_{c}")
                    nc.vector.tensor_mul(yt[:], ycp[:], rb[:])
                    y_sb[(p, c)] = yt

            # ---- partial projection + bias/2, ReduceScatter per 512-slice ----
            # bf16 partials/outputs: halves collective bytes; host casts back
            def emit_proj_slice(sl):
                part = dp.tile([QC, C], dt.bfloat16, name=f"part{sl}")
                for tt in range(4 * sl, 4 * sl + 4):
                    c = tt // 4
                    ps = pps.tile([128, 1024], dt.float32, tag="big", bufs=3)
                    for oc in range(2):
                        for p in range(NPACK):
                            nc.tensor.matmul(
                                ps[:, 512 * oc:512 * (oc + 1)],
                                lhsT=y_sb[(p, c)][:, 128 * (tt % 4):
                                                  128 * (tt % 4 + 1)],
                                rhs=wpT_sb[p][:, 512 * oc:512 * (oc + 1)],
                                start=(p == 0), stop=(p == 3),
                            )
                    os_ = op.tile([128, C], dt.bfloat16, tag="osb")
                    with nc.allow_low_precision("bf16 partial + pairwise reduce"):
                        nc.vector.tensor_add(os_[:], ps[:], bias_sb[:])
                    nc.sync.dma_start(
                        part[128 * (tt % 4):128 * (tt % 4 + 1), :], os_[:])
                rs_out = dp.tile([QC // 2, C], dt.bfloat16, name=f"rs{sl}")
                nc.gpsimd.collective_compute(
                    "ReduceScatter",
                    mybir.AluOpType.add,
                    replica_groups=[[0, 1], [2, 3], [4, 5], [6, 7]],
                    ins=[part[:]],
                    outs=[rs_out[:]],
                )
                nc.sync.dma_start(
                    out_ext[256 * sl:256 * (sl + 1), :], rs_out[:])

            # ---- emission schedule: QK0, V, then attention chunks with the
            #      next pack's QK quarters (or proj slices) interleaved ----
            # gap-filler blocks (V/QK/proj) are demoted in scheduler
            # priority so the attention S-matmul -> exp chain never starves;
            # emission ORDER still defines the data dependencies
            LOW = -1_000_000
            qk = {0: alloc_qk(0)}
            for q in (0, 2):        # Q half0, K half0: all chunks 0-1 need
                emit_qk_quarter(0, qk[0], q)
            # pack 0 (V interleaved), pack 1, then packs 2+3 chunk-interleaved
            # so proj slices (gated on pack 3's chunks) spread over the tail
            for c in range(NCHUNK):
                if c == 2:  # halves 1 needed from chunk 2 on
                    for q in (1, 3):
                        emit_qk_quarter(0, qk[0], q)
                with tc.high_priority(offset=LOW):
                    emit_v_quarter(c)
                emit_attention_chunk(0, *qk[0], c)
            qk[1] = alloc_qk(1)
            with tc.high_priority(offset=LOW):
                for q in (0, 2, 1, 3):
                    emit_qk_quarter(1, qk[1], q)
            for c in range(NCHUNK):
                emit_attention_chunk(1, *qk[1], c)
            for p in (2, 3):
                qk[p] = alloc_qk(p)
                with tc.high_priority(offset=LOW):
                    for q in (0, 2, 1, 3):
                        emit_qk_quarter(p, qk[p], q)
            for c in range(NCHUNK):
                emit_attention_chunk(2, *qk[2], c)
                emit_attention_chunk(3, *qk[3], c)
                with tc.high_priority(offset=LOW):
                    emit_proj_slice(c)

    nc.compile()
    return nc


_NC = None


def _get_nc():
    global _NC
    if _NC is None:
        _NC = build_nc()
    return _NC


def _make_in_maps(x, w_qkv, w_proj, b_proj):
    bf16 = ml_dtypes.bfloat16
    # causal masks for the 4 diagonal k-tiles of a 512-q chunk, both head
    # halves identical: mask_r[ki, qi] = qi >= 128*r + ki
    qi = np.arange(QC)[None, :]
    ki = np.arange(128)[:, None]
    mk = np.concatenate(
        [np.tile((qi >= 128 * r + ki), (1, 2)) for r in range(4)], axis=1)
    masks = mk.astype(bf16)
    ones = np.ones((128, 64), dtype=bf16)
    biasb = np.tile(b_proj.astype(np.float32)[None, :] * 0.5, (128, 1))

    in_maps = []
    for c in range(N_CORES):
        b, g = c // 2, c % 2
        xTc = np.ascontiguousarray(x[b].T).astype(bf16)
        rows = []
        for blk in range(3):  # q, k, v rows of w_qkv for this head group
            base = blk * C + g * GDIM
            rows.append(w_qkv[base:base + GDIM, :])
        wTc = np.ascontiguousarray(np.concatenate(rows, axis=0).T).astype(bf16)
        wpTc = np.ascontiguousarray(
            w_proj[:, g * GDIM:(g + 1) * GDIM].T).astype(bf16)
        in_maps.append({
            "xT": xTc, "wT": wTc, "wpT": wpTc,
            "biasb": biasb, "masks": masks, "ones": ones,
        })
    return in_maps


def kernel(x, w_qkv, w_proj, b_proj):
    x = np.asarray(x, dtype=np.float32)
    w_qkv = np.asarray(w_qkv, dtype=np.float32)
    w_proj = np.asarray(w_proj, dtype=np.float32)
    b_proj = np.asarray(b_proj, dtype=np.float32)

    nc = _get_nc()
    in_maps = _make_in_maps(x, w_qkv, w_proj, b_proj)
    res = run_bass_kernel_spmd(nc, in_maps, list(range(N_CORES)))

    # each 512-token-slice ReduceScatter scatters over the pair in rank
    # order (256 tokens each); host casts bf16 results back to f32
    out = np.empty((B, T, C), dtype=np.float32)
    for cc in range(N_CORES):
        b, g = cc // 2, cc % 2
        r = res.results[cc]["out_ext"].astype(np.float32)
        for sl in range(4):
            t0 = 512 * sl + 256 * g
            out[b, t0:t0 + 256, :] = r[256 * sl:256 * (sl + 1), :]
    return out
